# revision 15
# baseline (speedup 1.0000x reference)
"""Trainium2 Bass kernel for segment-wise Conv1d + ReLU + BatchNorm1d.

Reference computation (nn_ConvSeg):
  - x_all [32768, 256] fp32, segment_key [32768] sorted ids (<= 8 segments)
  - per-segment Conv1d (kernel K=9, zero padding 4 at segment boundaries)
  - ReLU, then BatchNorm1d over all tokens (training stats, biased var)

Strategy (v3):
  - Host inserts 4 zero rows at each segment boundary -> the ragged
    per-segment conv becomes ONE dense conv over the gapped sequence.
  - The gapped sequence (8*4104 positions) is split into 8 equal chunks
    (one per NeuronCore) with a 4-position halo on each side.
  - All streamed tensors (x, w, mask, out) are bf16: rel error ~3e-3 vs
    the 2e-2 gate, matmul throughput identical to fp32r (1 cycle/row),
    and DMA bytes halve. Stats/PSUM/normalize stay fp32.
  - Data is transposed to [d, position]; conv = 18 accumulation passes of
    128x128xBS bf16 matmuls per (block, out-chunk) into PSUM. Blocks are
    8x508 + 1x40 columns: the tiny last block shortens the post-conv
    stats chain that gates the collective.
  - ScalarE fuses bias + ReLU from PSUM; DVE does masked sum; ScalarE
    Square+accum gives sum-of-squares. Per-half st4 reductions are hoisted
    so only the tiny last block's stats sit on the critical path.
  - Cross-core BN stats use one AllGather (cheaper than AllReduce) of
    [128,4] sums; each core reduces the 8 gathered copies locally and
    finalizes with a single Rsqrt (one ACT table for the whole program).
  - Normalize writes bf16 and DMAs out in 3-block column groups; host
    upcasts, drops gap columns, reassembles [32768, 256] fp32.
"""

import numpy as np
from ml_dtypes import bfloat16

import concourse.bacc as bacc
import concourse.mybir as mybir
from concourse import tile
from concourse.bass_utils import run_bass_kernel_spmd

F32 = mybir.dt.float32
BF16 = mybir.dt.bfloat16
AF = mybir.ActivationFunctionType
OP = mybir.AluOpType
AX = mybir.AxisListType

N = 32768
D = 256  # d_in == d_out == 256
K = 9
PAD = K // 2
EPS = 1e-5

NCORES = 8
BSZ = [508] * 7 + [380, 104, 64]  # block widths (PSUM bank limit 512 fp32);
NB = len(BSZ)                     # shrinking tail so the last stats chain
OFF = [sum(BSZ[:b]) for b in range(NB)]  # (which gates the collective) is short
L = sum(BSZ)  # 4104 gapped positions per core
LH = L + 2 * PAD  # input columns incl. halo
GAP = 4  # zero rows inserted at each segment boundary (>= PAD)

# normalize / output column groups
_GB = [range(0, 3), range(3, 6), range(6, NB)]
GRP = [(OFF[bl[0]], OFF[bl[-1]] + BSZ[bl[-1]], bl) for bl in _GB]

_PROGRAM_CACHE: dict = {}


def build_program(repeat: int = 1, use_collective: bool = True,
                  conv_only: bool = False, collective_kind: str = "AllGather"):
    """Build + compile the SPMD Bass program (identical on all 8 cores)."""
    nc = bacc.Bacc(
        "TRN2", target_bir_lowering=False, debug=False, num_devices=NCORES
    )

    x_d = nc.declare_dram_parameter("x", [128, 2, LH], BF16, isOutput=False)
    w_d = nc.declare_dram_parameter("w", [128, 2, K * D], BF16, isOutput=False)
    m_d = nc.declare_dram_parameter("mask", [128, L], BF16, isOutput=False)
    bgb_d = nc.declare_dram_parameter("bgb", [128, 8], F32, isOutput=False)
    out_d = nc.declare_dram_parameter("out", [128, 2, L], BF16, isOutput=True)

    # x DMA chunks: a small first chunk so the PE can start early, then
    # three big ones. Block b only depends on chunk XBLK[b].
    XCH = [(0, OFF[0] + BSZ[0] + 2 * PAD),
           (OFF[0] + BSZ[0] + 2 * PAD, OFF[3] + BSZ[3] + 2 * PAD),
           (OFF[3] + BSZ[3] + 2 * PAD, OFF[6] + BSZ[6] + 2 * PAD),
           (OFF[6] + BSZ[6] + 2 * PAD, LH)]
    assert XCH[-1][0] < LH

    def stats_ops(b, oc, ysl, mt, scol, qcol, work, nc):
        j = oc * NB + b
        bs = BSZ[b]
        ym = work.tile([128, 508], F32, tag="ym")
        if b == NB - 1:
            # keep ACT (slow fixed-cost Square) off the critical tail:
            # mask-mult on Pool, sum + square-sum on DVE
            nc.gpsimd.tensor_tensor(
                ym[:, :bs], ysl, mt[:, OFF[b] : OFF[b] + bs], OP.mult
            )
            nc.vector.tensor_reduce(
                scol[:, j : j + 1], ym[:, :bs], AX.X, OP.add
            )
            sq = work.tile([128, 508], F32, tag="sq")
            nc.vector.tensor_tensor(sq[:, :bs], ym[:, :bs], ym[:, :bs],
                                    OP.mult)
            nc.vector.tensor_reduce(
                qcol[:, j : j + 1], sq[:, :bs], AX.X, OP.add
            )
            return
        nc.vector.tensor_tensor(
            ym[:, :bs], ysl, mt[:, OFF[b] : OFF[b] + bs], OP.mult
        )
        nc.vector.tensor_reduce(scol[:, j : j + 1], ym[:, :bs], AX.X, OP.add)
        sq = work.tile([128, 508], F32, tag="sq")
        nc.scalar.activation(
            sq[:, :bs], ym[:, :bs], AF.Square, bias=0.0, scale=1.0,
            accum_out=qcol[:, j : j + 1],
        )

    with tile.TileContext(nc) as tc:
        with (
            tc.tile_pool(name="const", bufs=1) as const,
            tc.tile_pool(name="ypool", bufs=1) as ypool,
            tc.tile_pool(name="psum", bufs=4, space="PSUM") as psum,
            tc.tile_pool(name="work", bufs=4) as work,
            tc.tile_pool(name="stats", bufs=1) as stats,
            tc.tile_pool(name="dram", bufs=2, space="DRAM") as dram,
        ):
            xt = const.tile([128, 2, LH], BF16)
            wt = const.tile([128, 2, K * D], BF16)
            mt = const.tile([128, L], BF16)
            bgbt = const.tile([128, 8], F32)
            ybig = ypool.tile([128, 2, L], F32)
            obig = ypool.tile([128, 2, L], BF16)
            scol = stats.tile([128, 2 * NB], F32)
            qcol = stats.tile([128, 2 * NB], F32)
            warm = stats.tile([128, 1], F32)

            for _ in range(repeat):
                # --- input DMAs, ordered so compute can start early ---
                lo, hi = XCH[0]
                nc.sync.dma_start(xt[:, :, lo:hi], x_d[:, :, lo:hi])
                nc.sync.dma_start(wt[:, :, 0:D], w_d[:, :, 0:D])
                nc.sync.dma_start(bgbt[:], bgb_d[:])
                # preload the single ACT table (relu/square/identity/sqrt)
                nc.scalar.activation(warm[:], bgbt[:, 6:7], AF.Sqrt)
                for k in range(1, K):  # remaining weights, one tap per DMA
                    nc.sync.dma_start(
                        wt[:, :, k * D : (k + 1) * D],
                        w_d[:, :, k * D : (k + 1) * D],
                    )
                for lo, hi in XCH[1:]:  # remaining x chunks
                    nc.sync.dma_start(xt[:, :, lo:hi], x_d[:, :, lo:hi])
                # mask, only needed by trailing stats ops
                nc.sync.dma_start(mt[:], m_d[:])

                # --- conv + relu + local BN stats ---
                st4 = stats.tile([128, 4], F32, tag="st4")
                for b in range(NB):
                    bs = BSZ[b]
                    for oc in range(2):
                        ps = psum.tile([128, 508], F32, tag="ps")
                        for k in range(K):
                            for dc in range(2):
                                nc.tensor.matmul(
                                    ps[:, :bs],
                                    wt[:, dc,
                                       k * D + oc * 128 : k * D + oc * 128 + 128],
                                    xt[:, dc, OFF[b] + k : OFF[b] + k + bs],
                                    start=(k == 0 and dc == 0),
                                    stop=(k == K - 1 and dc == 1),
                                )
                        ysl = ybig[:, oc, OFF[b] : OFF[b] + bs]
                        # y = relu(conv + bias)
                        if not conv_only and b == NB - 1:
                            # last block: relu on DVE straight from PSUM, so
                            # the tail chain has no ACT hop
                            nc.vector.tensor_scalar(
                                out=ysl, in0=ps[:, :bs],
                                scalar1=bgbt[:, oc : oc + 1], scalar2=0.0,
                                op0=OP.add, op1=OP.max,
                            )
                        else:
                            nc.scalar.activation(
                                ysl, ps[:, :bs], AF.Relu,
                                bias=bgbt[:, oc : oc + 1], scale=1.0,
                            )
                        if not conv_only:
                            stats_ops(b, oc, ysl, mt, scol, qcol, work, nc)
                        # st4 halves as soon as their last block is done, so
                        # only the tiny last block's chain gates the collective
                        if not conv_only and b == NB - 1:
                            nc.vector.tensor_reduce(
                                st4[:, oc : oc + 1],
                                scol[:, oc * NB : oc * NB + NB], AX.X, OP.add,
                            )
                            nc.vector.tensor_reduce(
                                st4[:, 2 + oc : 3 + oc],
                                qcol[:, oc * NB : oc * NB + NB], AX.X, OP.add,
                            )

                if conv_only:
                    for lo, hi, _ in GRP:
                        nc.sync.dma_start(
                            out_d[:, :, lo:hi], ybig[:, :, lo:hi]
                        )
                    continue

                # --- global BN stats (AllGather + local reduce) ---
                gst = stats.tile([128, 4], F32, tag="gst")
                if use_collective and collective_kind == "AllGather":
                    cc_in = dram.tile([128, 4], F32, tag="cc_in")
                    cc_out = dram.tile([8, 128, 4], F32, tag="cc_out")
                    nc.sync.dma_start(cc_in[:], st4[:])
                    nc.gpsimd.collective_compute(
                        "AllGather",
                        OP.bypass,
                        replica_groups=[list(range(NCORES))],
                        ins=[cc_in.opt()],
                        outs=[cc_out.opt()],
                    )
                    # land gathered stats transposed: [128, 8 cores, 4 stats]
                    g8 = stats.tile([128, 8, 4], F32, tag="g8")
                    nc.sync.dma_start(g8[:], cc_out[:].transpose([1, 0, 2]))
                    for s in range(4):
                        nc.vector.tensor_reduce(
                            gst[:, s : s + 1], g8[:, :, s], AX.X, OP.add
                        )
                elif use_collective:
                    cc_in = dram.tile([128, 4], F32, tag="cc_in")
                    cc_out = dram.tile([128, 4], F32, tag="cc_out")
                    nc.sync.dma_start(cc_in[:], st4[:])
                    nc.gpsimd.collective_compute(
                        "AllReduce",
                        OP.add,
                        replica_groups=[list(range(NCORES))],
                        ins=[cc_in.opt()],
                        outs=[cc_out.opt()],
                    )
                    nc.sync.dma_start(gst[:], cc_out[:])
                else:
                    nc.vector.tensor_scalar_mul(gst[:], st4[:], float(NCORES))

                # --- finalize: scale = gamma*rsqrt(var+eps),
                #     shift = beta - mean*scale ---
                m4 = stats.tile([128, 4], F32, tag="m4")  # [mean, E[y^2]]
                var = stats.tile([128, 2], F32, tag="var")
                std = stats.tile([128, 2], F32, tag="std")
                inv = stats.tile([128, 2], F32, tag="inv")
                scl = stats.tile([128, 2], F32, tag="scl")
                shf = stats.tile([128, 2], F32, tag="shf")
                nc.vector.tensor_scalar_mul(m4[:], gst[:], 1.0 / N)
                nc.vector.tensor_tensor(var[:], m4[:, 0:2], m4[:, 0:2], OP.mult)
                nc.vector.tensor_tensor(var[:], m4[:, 2:4], var[:], OP.subtract)
                nc.scalar.activation(
                    std[:], var[:], AF.Sqrt, bias=bgbt[:, 6:7], scale=1.0
                )
                nc.vector.reciprocal(inv[:], std[:])
                nc.vector.tensor_tensor(scl[:], bgbt[:, 2:4], inv[:], OP.mult)
                nc.vector.tensor_tensor(shf[:], m4[:, 0:2], scl[:], OP.mult)
                nc.vector.tensor_tensor(shf[:], bgbt[:, 4:6], shf[:], OP.subtract)

                # --- normalize (fp32 -> bf16) + write out in column groups ---
                for lo, hi, blocks in GRP:
                    half = (lo + hi) // 2
                    for oc in range(2):
                        # split each oc span between DVE and ACT/Pool so the
                        # three engines chew one group concurrently
                        nc.vector.tensor_scalar(
                            out=obig[:, oc, lo:half], in0=ybig[:, oc, lo:half],
                            scalar1=scl[:, oc : oc + 1],
                            scalar2=shf[:, oc : oc + 1],
                            op0=OP.mult, op1=OP.add,
                        )
                        if oc == 0:
                            nc.gpsimd.tensor_scalar(
                                out=obig[:, oc, half:hi],
                                in0=ybig[:, oc, half:hi],
                                scalar1=scl[:, oc : oc + 1],
                                scalar2=shf[:, oc : oc + 1],
                                op0=OP.mult, op1=OP.add,
                            )
                        else:
                            nc.scalar.activation(
                                obig[:, oc, half:hi], ybig[:, oc, half:hi],
                                AF.Identity,
                                bias=shf[:, oc : oc + 1],
                                scale=scl[:, oc : oc + 1],
                            )
                    nc.sync.dma_start(out_d[:, :, lo:hi], obig[:, :, lo:hi])

    nc.compile()
    return nc


def _get_program(repeat: int = 1, use_collective: bool = True):
    key = (repeat, use_collective)
    if key not in _PROGRAM_CACHE:
        _PROGRAM_CACHE[key] = build_program(repeat, use_collective)
    return _PROGRAM_CACHE[key]


def prepare_inputs(x_all, W, b, gamma, beta, segment_key):
    """Host-side sharding: gap insertion, transpose, per-core slicing.

    Returns (in_maps, tok_gpos) where tok_gpos[n] is the gapped position of
    token n in the concatenated per-core output space (core = pos // L).
    """
    x_all = np.ascontiguousarray(np.asarray(x_all, dtype=np.float32))
    W = np.asarray(W, dtype=np.float32)
    b = np.asarray(b, dtype=np.float32)
    gamma = np.asarray(gamma, dtype=np.float32)
    beta = np.asarray(beta, dtype=np.float32)
    seg = np.asarray(segment_key).reshape(-1)
    n = x_all.shape[0]
    assert n == N, f"kernel hardcodes N={N}, got {n}"

    # run-length segments of the sorted key
    change = np.flatnonzero(seg[1:] != seg[:-1]) + 1
    starts = np.concatenate(([0], change))
    ends = np.concatenate((change, [n]))
    nseg = len(starts)
    assert n + GAP * (nseg + 1) <= NCORES * L, "gapped sequence does not fit"

    # gapped position of each token
    tok_gpos = np.empty(n, dtype=np.int64)
    g = GAP
    for s, e in zip(starts, ends):
        tok_gpos[s:e] = g + np.arange(e - s)
        g += (e - s) + GAP

    # gapped, transposed input with halo: xg_t[:, PAD + gpos] = x_all[n]
    total = NCORES * L
    xg = np.zeros((total + 2 * PAD, D), dtype=np.float32)
    xg[PAD + tok_gpos] = x_all
    xg_t = np.ascontiguousarray(xg.T)  # [D, total + 2*PAD]

    mask = np.zeros(total, dtype=bfloat16)
    mask[tok_gpos] = 1.0

    # weights: wmat[d, k*D + o] = W[o, d, k] -> [128, 2, K*D] bf16
    wmat = W.transpose(1, 2, 0).reshape(D, K * D)
    w_in = np.ascontiguousarray(
        wmat.reshape(2, 128, K * D).transpose(1, 0, 2).astype(bfloat16)
    )

    eps_col = np.full(128, EPS, dtype=np.float32)
    zero_col = np.zeros(128, dtype=np.float32)
    bgb = np.stack(
        [b[:128], b[128:], gamma[:128], gamma[128:], beta[:128], beta[128:],
         eps_col, zero_col],
        axis=1,
    ).astype(np.float32)
    bgb = np.ascontiguousarray(bgb)

    in_maps = []
    for c in range(NCORES):
        xc = np.ascontiguousarray(
            xg_t[:, c * L : c * L + LH]
            .reshape(2, 128, LH).transpose(1, 0, 2).astype(bfloat16)
        )
        mc = np.ascontiguousarray(
            np.broadcast_to(mask[c * L : (c + 1) * L], (128, L))
        )
        in_maps.append({"x": xc, "w": w_in, "mask": mc, "bgb": bgb})
    return in_maps, tok_gpos


def assemble_output(results, tok_gpos):
    out = np.empty((N, D), dtype=np.float32)
    core = tok_gpos // L
    loc = tok_gpos % L
    for c in range(NCORES):
        sel = core == c
        # [128, 2, L] bf16 -> [256, L] fp32
        full = (
            results[c]["out"].astype(np.float32)
            .transpose(1, 0, 2).reshape(D, L)
        )
        out[sel] = full[:, loc[sel]].T
    return out


def kernel(x_all, W, b, gamma, beta, segment_key):
    nc = _get_program()
    in_maps, tok_gpos = prepare_inputs(x_all, W, b, gamma, beta, segment_key)
    res = run_bass_kernel_spmd(nc, in_maps, list(range(NCORES)))
    return assemble_output(res.results, tok_gpos)


# revision 16
# speedup vs baseline: 1.0089x; 1.0089x over previous
"""Trainium2 Bass kernel for segment-wise Conv1d + ReLU + BatchNorm1d.

Reference computation (nn_ConvSeg):
  - x_all [32768, 256] fp32, segment_key [32768] sorted ids (<= 8 segments)
  - per-segment Conv1d (kernel K=9, zero padding 4 at segment boundaries)
  - ReLU, then BatchNorm1d over all tokens (training stats, biased var)

Strategy (v3):
  - Host inserts 4 zero rows at each segment boundary -> the ragged
    per-segment conv becomes ONE dense conv over the gapped sequence.
  - The gapped sequence (8*4104 positions) is split into 8 equal chunks
    (one per NeuronCore) with a 4-position halo on each side.
  - All streamed tensors (x, w, mask, out) are bf16: rel error ~3e-3 vs
    the 2e-2 gate, matmul throughput identical to fp32r (1 cycle/row),
    and DMA bytes halve. Stats/PSUM/normalize stay fp32.
  - Data is transposed to [d, position]; conv = 18 accumulation passes of
    128x128xBS bf16 matmuls per (block, out-chunk) into PSUM. Blocks are
    8x508 + 1x40 columns: the tiny last block shortens the post-conv
    stats chain that gates the collective.
  - ScalarE fuses bias + ReLU from PSUM; DVE does masked sum; ScalarE
    Square+accum gives sum-of-squares. Per-half st4 reductions are hoisted
    so only the tiny last block's stats sit on the critical path.
  - Cross-core BN stats use one AllGather (cheaper than AllReduce) of
    [128,4] sums; each core reduces the 8 gathered copies locally and
    finalizes with a single Rsqrt (one ACT table for the whole program).
  - Normalize writes bf16 and DMAs out in 3-block column groups; host
    upcasts, drops gap columns, reassembles [32768, 256] fp32.
"""

import numpy as np
from ml_dtypes import bfloat16

import concourse.bacc as bacc
import concourse.mybir as mybir
from concourse import tile
from concourse.bass_utils import run_bass_kernel_spmd

F32 = mybir.dt.float32
BF16 = mybir.dt.bfloat16
AF = mybir.ActivationFunctionType
OP = mybir.AluOpType
AX = mybir.AxisListType

N = 32768
D = 256  # d_in == d_out == 256
K = 9
PAD = K // 2
EPS = 1e-5

NCORES = 8
BSZ = [508] * 7 + [380, 104, 64]  # block widths (PSUM bank limit 512 fp32);
NB = len(BSZ)                     # shrinking tail so the last stats chain
OFF = [sum(BSZ[:b]) for b in range(NB)]  # (which gates the collective) is short
L = sum(BSZ)  # 4104 gapped positions per core
LH = L + 2 * PAD  # input columns incl. halo
GAP = 4  # zero rows inserted at each segment boundary (>= PAD)

# normalize / output column groups
_GB = [range(0, 3), range(3, 6), range(6, NB)]
GRP = [(OFF[bl[0]], OFF[bl[-1]] + BSZ[bl[-1]], bl) for bl in _GB]

_PROGRAM_CACHE: dict = {}


def build_program(repeat: int = 1, use_collective: bool = True,
                  conv_only: bool = False, collective_kind: str = "AllGather"):
    """Build + compile the SPMD Bass program (identical on all 8 cores)."""
    nc = bacc.Bacc(
        "TRN2", target_bir_lowering=False, debug=False, num_devices=NCORES
    )

    x_d = nc.declare_dram_parameter("x", [128, 2, LH], BF16, isOutput=False)
    w_d = nc.declare_dram_parameter("w", [128, 2, K * D], BF16, isOutput=False)
    m_d = nc.declare_dram_parameter("mask", [128, L], BF16, isOutput=False)
    bgb_d = nc.declare_dram_parameter("bgb", [128, 8], F32, isOutput=False)
    out_d = nc.declare_dram_parameter("out", [128, 2, L], BF16, isOutput=True)

    # x DMA chunks: a small first chunk so the PE can start early, then
    # three big ones. Block b only depends on chunk XBLK[b].
    XCH = [(0, OFF[0] + BSZ[0] + 2 * PAD),
           (OFF[0] + BSZ[0] + 2 * PAD, OFF[3] + BSZ[3] + 2 * PAD),
           (OFF[3] + BSZ[3] + 2 * PAD, OFF[6] + BSZ[6] + 2 * PAD),
           (OFF[6] + BSZ[6] + 2 * PAD, LH)]
    assert XCH[-1][0] < LH

    def stats_ops(b, oc, ysl, mt, scol, qcol, work, nc):
        j = oc * NB + b
        bs = BSZ[b]
        ym = work.tile([128, 508], F32, tag="ym")
        if b == NB - 1:
            # keep ACT (slow fixed-cost Square) off the critical tail:
            # mask-mult on Pool, sum + square-sum on DVE
            nc.gpsimd.tensor_tensor(
                ym[:, :bs], ysl, mt[:, OFF[b] : OFF[b] + bs], OP.mult
            )
            nc.vector.tensor_reduce(
                scol[:, j : j + 1], ym[:, :bs], AX.X, OP.add
            )
            sq = work.tile([128, 508], F32, tag="sq")
            nc.vector.tensor_tensor(sq[:, :bs], ym[:, :bs], ym[:, :bs],
                                    OP.mult)
            nc.vector.tensor_reduce(
                qcol[:, j : j + 1], sq[:, :bs], AX.X, OP.add
            )
            return
        nc.vector.tensor_tensor(
            ym[:, :bs], ysl, mt[:, OFF[b] : OFF[b] + bs], OP.mult
        )
        nc.vector.tensor_reduce(scol[:, j : j + 1], ym[:, :bs], AX.X, OP.add)
        sq = work.tile([128, 508], F32, tag="sq")
        nc.scalar.activation(
            sq[:, :bs], ym[:, :bs], AF.Square, bias=0.0, scale=1.0,
            accum_out=qcol[:, j : j + 1],
        )

    with tile.TileContext(nc) as tc:
        with (
            tc.tile_pool(name="const", bufs=1) as const,
            tc.tile_pool(name="ypool", bufs=1) as ypool,
            tc.tile_pool(name="psum", bufs=4, space="PSUM") as psum,
            tc.tile_pool(name="work", bufs=4) as work,
            tc.tile_pool(name="stats", bufs=1) as stats,
            tc.tile_pool(name="dram", bufs=2, space="DRAM") as dram,
        ):
            xt = const.tile([128, 2, LH], BF16)
            wt = const.tile([128, 2, K * D], BF16)
            mt = const.tile([128, L], BF16)
            bgbt = const.tile([128, 8], F32)
            ybig = ypool.tile([128, 2, L], F32)
            obig = ypool.tile([128, 2, L], BF16)
            scol = stats.tile([128, 2 * NB], F32)
            qcol = stats.tile([128, 2 * NB], F32)
            warm = stats.tile([128, 1], F32)

            for _ in range(repeat):
                # --- input DMAs, ordered so compute can start early ---
                lo, hi = XCH[0]
                nc.sync.dma_start(xt[:, :, lo:hi], x_d[:, :, lo:hi])
                nc.sync.dma_start(wt[:, :, 0:D], w_d[:, :, 0:D])
                nc.sync.dma_start(bgbt[:], bgb_d[:])
                # preload the single ACT table (relu/square/identity/sqrt)
                nc.scalar.activation(warm[:], bgbt[:, 6:7], AF.Sqrt)
                for k in range(1, K):  # remaining weights, one tap per DMA
                    nc.sync.dma_start(
                        wt[:, :, k * D : (k + 1) * D],
                        w_d[:, :, k * D : (k + 1) * D],
                    )
                for lo, hi in XCH[1:]:  # remaining x chunks
                    nc.sync.dma_start(xt[:, :, lo:hi], x_d[:, :, lo:hi])
                # mask, only needed by trailing stats ops
                nc.sync.dma_start(mt[:], m_d[:])

                # --- conv + relu + local BN stats ---
                st4 = stats.tile([128, 4], F32, tag="st4")
                for b in range(NB):
                    bs = BSZ[b]
                    for oc in range(2):
                        ps = psum.tile([128, 508], F32, tag="ps")
                        for k in range(K):
                            for dc in range(2):
                                nc.tensor.matmul(
                                    ps[:, :bs],
                                    wt[:, dc,
                                       k * D + oc * 128 : k * D + oc * 128 + 128],
                                    xt[:, dc, OFF[b] + k : OFF[b] + k + bs],
                                    start=(k == 0 and dc == 0),
                                    stop=(k == K - 1 and dc == 1),
                                )
                        ysl = ybig[:, oc, OFF[b] : OFF[b] + bs]
                        # y = relu(conv + bias)
                        if not conv_only and b == NB - 1:
                            # last block: relu on DVE straight from PSUM, so
                            # the tail chain has no ACT hop
                            nc.vector.tensor_scalar(
                                out=ysl, in0=ps[:, :bs],
                                scalar1=bgbt[:, oc : oc + 1], scalar2=0.0,
                                op0=OP.add, op1=OP.max,
                            )
                        else:
                            nc.scalar.activation(
                                ysl, ps[:, :bs], AF.Relu,
                                bias=bgbt[:, oc : oc + 1], scale=1.0,
                            )
                        if not conv_only:
                            stats_ops(b, oc, ysl, mt, scol, qcol, work, nc)
                        # st4 halves as soon as their last block is done, so
                        # only the tiny last block's chain gates the collective
                        if not conv_only and b == NB - 1:
                            nc.vector.tensor_reduce(
                                st4[:, oc : oc + 1],
                                scol[:, oc * NB : oc * NB + NB], AX.X, OP.add,
                            )
                            nc.vector.tensor_reduce(
                                st4[:, 2 + oc : 3 + oc],
                                qcol[:, oc * NB : oc * NB + NB], AX.X, OP.add,
                            )

                if conv_only:
                    for lo, hi, _ in GRP:
                        nc.sync.dma_start(
                            out_d[:, :, lo:hi], ybig[:, :, lo:hi]
                        )
                    continue

                # --- global BN stats (AllGather + local reduce) ---
                gst = stats.tile([128, 4], F32, tag="gst")
                if use_collective and collective_kind == "AllGather":
                    cc_in = dram.tile([128, 4], F32, tag="cc_in")
                    cc_out = dram.tile([8, 128, 4], F32, tag="cc_out")
                    nc.sync.dma_start(cc_in[:], st4[:])
                    nc.gpsimd.collective_compute(
                        "AllGather",
                        OP.bypass,
                        replica_groups=[list(range(NCORES))],
                        ins=[cc_in.opt()],
                        outs=[cc_out.opt()],
                    )
                    # land gathered stats transposed: [128, 8 cores, 4 stats]
                    # issued from Pool: FIFO after the collective, no sem hop
                    g8 = stats.tile([128, 8, 4], F32, tag="g8")
                    nc.gpsimd.dma_start(g8[:], cc_out[:].transpose([1, 0, 2]))
                    for s in range(4):
                        nc.vector.tensor_reduce(
                            gst[:, s : s + 1], g8[:, :, s], AX.X, OP.add
                        )
                elif use_collective:
                    cc_in = dram.tile([128, 4], F32, tag="cc_in")
                    cc_out = dram.tile([128, 4], F32, tag="cc_out")
                    nc.sync.dma_start(cc_in[:], st4[:])
                    nc.gpsimd.collective_compute(
                        "AllReduce",
                        OP.add,
                        replica_groups=[list(range(NCORES))],
                        ins=[cc_in.opt()],
                        outs=[cc_out.opt()],
                    )
                    nc.sync.dma_start(gst[:], cc_out[:])
                else:
                    nc.vector.tensor_scalar_mul(gst[:], st4[:], float(NCORES))

                # --- finalize: scale = gamma*rsqrt(var+eps),
                #     shift = beta - mean*scale ---
                m4 = stats.tile([128, 4], F32, tag="m4")  # [mean, E[y^2]]
                var = stats.tile([128, 2], F32, tag="var")
                std = stats.tile([128, 2], F32, tag="std")
                inv = stats.tile([128, 2], F32, tag="inv")
                scl = stats.tile([128, 2], F32, tag="scl")
                shf = stats.tile([128, 2], F32, tag="shf")
                nc.vector.tensor_scalar_mul(m4[:], gst[:], 1.0 / N)
                nc.vector.tensor_tensor(var[:], m4[:, 0:2], m4[:, 0:2], OP.mult)
                nc.vector.tensor_tensor(var[:], m4[:, 2:4], var[:], OP.subtract)
                nc.scalar.activation(
                    std[:], var[:], AF.Sqrt, bias=bgbt[:, 6:7], scale=1.0
                )
                nc.vector.reciprocal(inv[:], std[:])
                nc.vector.tensor_tensor(scl[:], bgbt[:, 2:4], inv[:], OP.mult)
                nc.vector.tensor_tensor(shf[:], m4[:, 0:2], scl[:], OP.mult)
                nc.vector.tensor_tensor(shf[:], bgbt[:, 4:6], shf[:], OP.subtract)

                # --- normalize (fp32 -> bf16) + write out in column groups ---
                for lo, hi, blocks in GRP:
                    half = (lo + hi) // 2
                    for oc in range(2):
                        # split each oc span between DVE and ACT/Pool so the
                        # three engines chew one group concurrently
                        nc.vector.tensor_scalar(
                            out=obig[:, oc, lo:half], in0=ybig[:, oc, lo:half],
                            scalar1=scl[:, oc : oc + 1],
                            scalar2=shf[:, oc : oc + 1],
                            op0=OP.mult, op1=OP.add,
                        )
                        if oc == 0:
                            nc.gpsimd.tensor_scalar(
                                out=obig[:, oc, half:hi],
                                in0=ybig[:, oc, half:hi],
                                scalar1=scl[:, oc : oc + 1],
                                scalar2=shf[:, oc : oc + 1],
                                op0=OP.mult, op1=OP.add,
                            )
                        else:
                            nc.scalar.activation(
                                obig[:, oc, half:hi], ybig[:, oc, half:hi],
                                AF.Identity,
                                bias=shf[:, oc : oc + 1],
                                scale=scl[:, oc : oc + 1],
                            )
                    nc.sync.dma_start(out_d[:, :, lo:hi], obig[:, :, lo:hi])

    nc.compile()
    return nc


def _get_program(repeat: int = 1, use_collective: bool = True):
    key = (repeat, use_collective)
    if key not in _PROGRAM_CACHE:
        _PROGRAM_CACHE[key] = build_program(repeat, use_collective)
    return _PROGRAM_CACHE[key]


def prepare_inputs(x_all, W, b, gamma, beta, segment_key):
    """Host-side sharding: gap insertion, transpose, per-core slicing.

    Returns (in_maps, tok_gpos) where tok_gpos[n] is the gapped position of
    token n in the concatenated per-core output space (core = pos // L).
    """
    x_all = np.ascontiguousarray(np.asarray(x_all, dtype=np.float32))
    W = np.asarray(W, dtype=np.float32)
    b = np.asarray(b, dtype=np.float32)
    gamma = np.asarray(gamma, dtype=np.float32)
    beta = np.asarray(beta, dtype=np.float32)
    seg = np.asarray(segment_key).reshape(-1)
    n = x_all.shape[0]
    assert n == N, f"kernel hardcodes N={N}, got {n}"

    # run-length segments of the sorted key
    change = np.flatnonzero(seg[1:] != seg[:-1]) + 1
    starts = np.concatenate(([0], change))
    ends = np.concatenate((change, [n]))
    nseg = len(starts)
    assert n + GAP * (nseg + 1) <= NCORES * L, "gapped sequence does not fit"

    # gapped position of each token
    tok_gpos = np.empty(n, dtype=np.int64)
    g = GAP
    for s, e in zip(starts, ends):
        tok_gpos[s:e] = g + np.arange(e - s)
        g += (e - s) + GAP

    # gapped, transposed input with halo: xg_t[:, PAD + gpos] = x_all[n]
    total = NCORES * L
    xg = np.zeros((total + 2 * PAD, D), dtype=np.float32)
    xg[PAD + tok_gpos] = x_all
    xg_t = np.ascontiguousarray(xg.T)  # [D, total + 2*PAD]

    mask = np.zeros(total, dtype=bfloat16)
    mask[tok_gpos] = 1.0

    # weights: wmat[d, k*D + o] = W[o, d, k] -> [128, 2, K*D] bf16
    wmat = W.transpose(1, 2, 0).reshape(D, K * D)
    w_in = np.ascontiguousarray(
        wmat.reshape(2, 128, K * D).transpose(1, 0, 2).astype(bfloat16)
    )

    eps_col = np.full(128, EPS, dtype=np.float32)
    zero_col = np.zeros(128, dtype=np.float32)
    bgb = np.stack(
        [b[:128], b[128:], gamma[:128], gamma[128:], beta[:128], beta[128:],
         eps_col, zero_col],
        axis=1,
    ).astype(np.float32)
    bgb = np.ascontiguousarray(bgb)

    in_maps = []
    for c in range(NCORES):
        xc = np.ascontiguousarray(
            xg_t[:, c * L : c * L + LH]
            .reshape(2, 128, LH).transpose(1, 0, 2).astype(bfloat16)
        )
        mc = np.ascontiguousarray(
            np.broadcast_to(mask[c * L : (c + 1) * L], (128, L))
        )
        in_maps.append({"x": xc, "w": w_in, "mask": mc, "bgb": bgb})
    return in_maps, tok_gpos


def assemble_output(results, tok_gpos):
    out = np.empty((N, D), dtype=np.float32)
    core = tok_gpos // L
    loc = tok_gpos % L
    for c in range(NCORES):
        sel = core == c
        # [128, 2, L] bf16 -> [256, L] fp32
        full = (
            results[c]["out"].astype(np.float32)
            .transpose(1, 0, 2).reshape(D, L)
        )
        out[sel] = full[:, loc[sel]].T
    return out


def kernel(x_all, W, b, gamma, beta, segment_key):
    nc = _get_program()
    in_maps, tok_gpos = prepare_inputs(x_all, W, b, gamma, beta, segment_key)
    res = run_bass_kernel_spmd(nc, in_maps, list(range(NCORES)))
    return assemble_output(res.results, tok_gpos)


# revision 24
# speedup vs baseline: 1.0545x; 1.0452x over previous
"""Trainium2 Bass kernel for segment-wise Conv1d + ReLU + BatchNorm1d.

Reference computation (nn_ConvSeg):
  - x_all [32768, 256] fp32, segment_key [32768] sorted ids (<= 8 segments)
  - per-segment Conv1d (kernel K=9, zero padding 4 at segment boundaries)
  - ReLU, then BatchNorm1d over all tokens (training stats, biased var)

Strategy (v3):
  - Host inserts 4 zero rows at each segment boundary -> the ragged
    per-segment conv becomes ONE dense conv over the gapped sequence.
  - The gapped sequence (8*4104 positions) is split into 8 equal chunks
    (one per NeuronCore) with a 4-position halo on each side.
  - All streamed tensors (x, w, mask, out) are bf16: rel error ~3e-3 vs
    the 2e-2 gate, matmul throughput identical to fp32r (1 cycle/row),
    and DMA bytes halve. Stats/PSUM/normalize stay fp32.
  - Data is transposed to [d, position]; conv = 18 accumulation passes of
    128x128xBS bf16 matmuls per (block, out-chunk) into PSUM. Blocks are
    8x508 + 1x40 columns: the tiny last block shortens the post-conv
    stats chain that gates the collective.
  - ScalarE fuses bias + ReLU from PSUM; DVE does masked sum; ScalarE
    Square+accum gives sum-of-squares. Per-half st4 reductions are hoisted
    so only the tiny last block's stats sit on the critical path.
  - Cross-core BN stats use one AllGather (cheaper than AllReduce) of
    [128,4] sums; each core reduces the 8 gathered copies locally and
    finalizes with Sqrt+reciprocal (one ACT table for the whole program).
  - Normalize writes bf16 and DMAs out in 3-block column groups; host
    upcasts, drops gap columns, reassembles [32768, 256] fp32.
"""

import numpy as np
from ml_dtypes import bfloat16

import concourse.bacc as bacc
import concourse.mybir as mybir
from concourse import tile
from concourse.bass_utils import run_bass_kernel_spmd

F32 = mybir.dt.float32
BF16 = mybir.dt.bfloat16
AF = mybir.ActivationFunctionType
OP = mybir.AluOpType
AX = mybir.AxisListType

N = 32768
D = 256  # d_in == d_out == 256
K = 9
PAD = K // 2
EPS = 1e-5

NCORES = 8
BSZ = [508] * 7 + [380, 104, 64]  # block widths (PSUM bank limit 512 fp32);
NB = len(BSZ)                     # shrinking tail so the last stats chain
OFF = [sum(BSZ[:b]) for b in range(NB)]  # (which gates the collective) is short
L = sum(BSZ)  # 4104 gapped positions per core
LH = L + 2 * PAD  # input columns incl. halo
GAP = 4  # zero rows inserted at each segment boundary (>= PAD)

# normalize / output column groups
_GB = [range(0, 3), range(3, 6), range(6, NB)]
GRP = [(OFF[bl[0]], OFF[bl[-1]] + BSZ[bl[-1]], bl) for bl in _GB]

_PROGRAM_CACHE: dict = {}


def build_program(repeat: int = 1, use_collective: bool = True,
                  conv_only: bool = False, collective_kind: str = "AllGather"):
    """Build + compile the SPMD Bass program (identical on all 8 cores)."""
    nc = bacc.Bacc(
        "TRN2", target_bir_lowering=False, debug=False, num_devices=NCORES
    )

    x_d = nc.declare_dram_parameter("x", [128, 2, LH], BF16, isOutput=False)
    w_d = nc.declare_dram_parameter("w", [128, 2, K * D], BF16, isOutput=False)
    m_d = nc.declare_dram_parameter("mask", [128, L], BF16, isOutput=False)
    bgb_d = nc.declare_dram_parameter("bgb", [128, 8], F32, isOutput=False)
    out_d = nc.declare_dram_parameter("out", [128, 2, L], BF16, isOutput=True)

    # x DMA chunks: a small first chunk so the PE can start early, then
    # three big ones. Block b only depends on chunk XBLK[b].
    XCH = [(0, OFF[0] + BSZ[0] + 2 * PAD),
           (OFF[0] + BSZ[0] + 2 * PAD, OFF[3] + BSZ[3] + 2 * PAD),
           (OFF[3] + BSZ[3] + 2 * PAD, OFF[6] + BSZ[6] + 2 * PAD),
           (OFF[6] + BSZ[6] + 2 * PAD, LH)]
    assert XCH[-1][0] < LH

    def stats_ops(b, oc, ysl, mt, scol, qcol, work, nc):  # noqa: C901
        j = oc * NB + b
        bs = BSZ[b]
        ym = work.tile([128, 508], F32, tag="ym")
        if b == NB - 1 and oc == 1:
            # keep ACT (slow fixed-cost Square) off the critical tail:
            # mask-mult on Pool, sum + square-sum on DVE
            nc.gpsimd.tensor_tensor(
                ym[:, :bs], ysl, mt[:, OFF[b] : OFF[b] + bs], OP.mult
            )
            nc.vector.tensor_reduce(
                scol[:, j : j + 1], ym[:, :bs], AX.X, OP.add
            )
            sq = work.tile([128, 508], F32, tag="sq")
            nc.vector.tensor_tensor(sq[:, :bs], ym[:, :bs], ym[:, :bs],
                                    OP.mult)
            nc.vector.tensor_reduce(
                qcol[:, j : j + 1], sq[:, :bs], AX.X, OP.add
            )
            return
        nc.vector.tensor_tensor(
            ym[:, :bs], ysl, mt[:, OFF[b] : OFF[b] + bs], OP.mult
        )
        nc.vector.tensor_reduce(scol[:, j : j + 1], ym[:, :bs], AX.X, OP.add)
        sq = work.tile([128, 508], F32, tag="sq")
        nc.scalar.activation(
            sq[:, :bs], ym[:, :bs], AF.Square, bias=0.0, scale=1.0,
            accum_out=qcol[:, j : j + 1],
        )

    with tile.TileContext(nc) as tc:
        with (
            tc.tile_pool(name="const", bufs=1) as const,
            tc.tile_pool(name="ypool", bufs=1) as ypool,
            tc.tile_pool(name="psum", bufs=4, space="PSUM") as psum,
            tc.tile_pool(name="work", bufs=4) as work,
            tc.tile_pool(name="stats", bufs=1) as stats,
            tc.tile_pool(name="dram", bufs=2, space="DRAM") as dram,
        ):
            xt = const.tile([128, 2, LH], BF16)
            wt = const.tile([128, 2, K * D], BF16)
            mt = const.tile([128, L], BF16)
            bgbt = const.tile([128, 8], F32)
            ybig = ypool.tile([128, 2, L], F32)
            obig = ypool.tile([128, 2, L], BF16)
            scol = stats.tile([128, 2 * NB], F32)
            qcol = stats.tile([128, 2 * NB], F32)
            warm = stats.tile([128, 1], F32)

            for _ in range(repeat):
                # --- input DMAs, ordered so compute can start early ---
                lo, hi = XCH[0]
                nc.sync.dma_start(xt[:, :, lo:hi], x_d[:, :, lo:hi])
                nc.sync.dma_start(wt[:, :, 0:D], w_d[:, :, 0:D])
                nc.sync.dma_start(bgbt[:], bgb_d[:])
                # preload the single ACT table (relu/square/identity/sqrt)
                nc.scalar.activation(warm[:], bgbt[:, 6:7], AF.Sqrt)
                for k in range(1, K):  # remaining weights, one tap per DMA
                    nc.sync.dma_start(
                        wt[:, :, k * D : (k + 1) * D],
                        w_d[:, :, k * D : (k + 1) * D],
                    )
                for lo, hi in XCH[1:]:  # remaining x chunks
                    nc.sync.dma_start(xt[:, :, lo:hi], x_d[:, :, lo:hi])
                # mask, only needed by trailing stats ops
                nc.sync.dma_start(mt[:], m_d[:])

                # --- conv + relu + local BN stats ---
                st4 = stats.tile([128, 4], F32, tag="st4")
                for b in range(NB):
                    bs = BSZ[b]
                    for oc in range(2):
                        ps = psum.tile([128, 508], F32, tag="ps")
                        for k in range(K):
                            for dc in range(2):
                                nc.tensor.matmul(
                                    ps[:, :bs],
                                    wt[:, dc,
                                       k * D + oc * 128 : k * D + oc * 128 + 128],
                                    xt[:, dc, OFF[b] + k : OFF[b] + k + bs],
                                    start=(k == 0 and dc == 0),
                                    stop=(k == K - 1 and dc == 1),
                                )
                        ysl = ybig[:, oc, OFF[b] : OFF[b] + bs]
                        # y = relu(conv + bias)
                        if not conv_only and b == NB - 1 and oc == 1:
                            # last block: relu on DVE straight from PSUM, so
                            # the tail chain has no ACT hop
                            nc.vector.tensor_scalar(
                                out=ysl, in0=ps[:, :bs],
                                scalar1=bgbt[:, oc : oc + 1], scalar2=0.0,
                                op0=OP.add, op1=OP.max,
                            )
                        else:
                            nc.scalar.activation(
                                ysl, ps[:, :bs], AF.Relu,
                                bias=bgbt[:, oc : oc + 1], scale=1.0,
                            )
                        if not conv_only:
                            stats_ops(b, oc, ysl, mt, scol, qcol, work, nc)
                        # st4 halves as soon as their last block is done, so
                        # only the tiny last block's chain gates the collective
                        if not conv_only and b == NB - 1:
                            nc.vector.tensor_reduce(
                                st4[:, oc : oc + 1],
                                scol[:, oc * NB : oc * NB + NB], AX.X, OP.add,
                            )
                            nc.vector.tensor_reduce(
                                st4[:, 2 + oc : 3 + oc],
                                qcol[:, oc * NB : oc * NB + NB], AX.X, OP.add,
                            )

                if conv_only:
                    for lo, hi, _ in GRP:
                        nc.sync.dma_start(
                            out_d[:, :, lo:hi], ybig[:, :, lo:hi]
                        )
                    continue

                # --- global BN stats (AllGather + local reduce) ---
                gst = stats.tile([128, 4], F32, tag="gst")
                if use_collective and collective_kind == "AllGather":
                    cc_in = dram.tile([128, 4], F32, tag="cc_in")
                    cc_out = dram.tile([8, 128, 4], F32, tag="cc_out")
                    # Pool-issued: the collective is the next Pool op, so no
                    # cross-engine completion hop
                    nc.gpsimd.dma_start(cc_in[:], st4[:])
                    nc.gpsimd.collective_compute(
                        "AllGather",
                        OP.bypass,
                        replica_groups=[list(range(NCORES))],
                        ins=[cc_in.opt()],
                        outs=[cc_out.opt()],
                    )
                    # land gathered stats transposed: [128, 8 cores, 4 stats]
                    # issued from Pool: FIFO after the collective, no sem hop
                    g8 = stats.tile([128, 8, 4], F32, tag="g8")
                    nc.gpsimd.dma_start(g8[:], cc_out[:].transpose([1, 0, 2]))
                    # tree-add the 8 copies on Pool itself (same engine as
                    # the DMA -> only the DMA sem, no second engine wake-up)
                    h4 = stats.tile([128, 4, 4], F32, tag="h4")
                    h2 = stats.tile([128, 2, 4], F32, tag="h2")
                    nc.gpsimd.tensor_tensor(
                        h4[:], g8[:, 0:4, :], g8[:, 4:8, :], OP.add
                    )
                    nc.gpsimd.tensor_tensor(
                        h2[:], h4[:, 0:2, :], h4[:, 2:4, :], OP.add
                    )
                    nc.gpsimd.tensor_tensor(
                        gst[:], h2[:, 0, :], h2[:, 1, :], OP.add
                    )
                elif use_collective:
                    cc_in = dram.tile([128, 4], F32, tag="cc_in")
                    cc_out = dram.tile([128, 4], F32, tag="cc_out")
                    nc.sync.dma_start(cc_in[:], st4[:])
                    nc.gpsimd.collective_compute(
                        "AllReduce",
                        OP.add,
                        replica_groups=[list(range(NCORES))],
                        ins=[cc_in.opt()],
                        outs=[cc_out.opt()],
                    )
                    nc.sync.dma_start(gst[:], cc_out[:])
                else:
                    nc.vector.tensor_scalar_mul(gst[:], st4[:], float(NCORES))

                # --- finalize: scale = gamma*rsqrt(var+eps),
                #     shift = beta - mean*scale ---
                m4 = stats.tile([128, 4], F32, tag="m4")  # [mean, E[y^2]]
                var = stats.tile([128, 2], F32, tag="var")
                std = stats.tile([128, 2], F32, tag="std")
                inv = stats.tile([128, 2], F32, tag="inv")
                scl = stats.tile([128, 2], F32, tag="scl")
                shf = stats.tile([128, 2], F32, tag="shf")
                nc.gpsimd.tensor_scalar_mul(m4[:], gst[:], 1.0 / N)
                nc.gpsimd.tensor_tensor(var[:], m4[:, 0:2], m4[:, 0:2], OP.mult)
                nc.gpsimd.tensor_tensor(var[:], m4[:, 2:4], var[:], OP.subtract)
                nc.scalar.activation(
                    std[:], var[:], AF.Sqrt, bias=bgbt[:, 6:7], scale=1.0
                )
                nc.vector.reciprocal(inv[:], std[:])
                nc.vector.tensor_tensor(scl[:], bgbt[:, 2:4], inv[:], OP.mult)
                nc.vector.tensor_tensor(shf[:], m4[:, 0:2], scl[:], OP.mult)
                nc.vector.tensor_tensor(shf[:], bgbt[:, 4:6], shf[:], OP.subtract)

                # --- normalize (fp32 -> bf16) + write out in column groups ---
                first = True
                for lo, hi, blocks in GRP:
                    # engine shares by throughput: DVE 0.6 / Pool 0.835 /
                    # ACT 1.08 ns per column
                    W = hi - lo
                    d0 = lo + (3 * W) // 4          # oc0: DVE then Pool
                    d1 = lo + (13 * W) // 100       # oc1: DVE, Pool, ACT
                    p1 = lo + (51 * W) // 100
                    nc.vector.tensor_scalar(
                        out=obig[:, 0, lo:d0], in0=ybig[:, 0, lo:d0],
                        scalar1=scl[:, 0:1], scalar2=shf[:, 0:1],
                        op0=OP.mult, op1=OP.add,
                    )
                    nc.gpsimd.tensor_scalar(
                        out=obig[:, 0, d0:hi], in0=ybig[:, 0, d0:hi],
                        scalar1=scl[:, 0:1], scalar2=shf[:, 0:1],
                        op0=OP.mult, op1=OP.add,
                    )
                    nc.vector.tensor_scalar(
                        out=obig[:, 1, lo:d1], in0=ybig[:, 1, lo:d1],
                        scalar1=scl[:, 1:2], scalar2=shf[:, 1:2],
                        op0=OP.mult, op1=OP.add,
                    )
                    nc.gpsimd.tensor_scalar(
                        out=obig[:, 1, d1:p1], in0=ybig[:, 1, d1:p1],
                        scalar1=scl[:, 1:2], scalar2=shf[:, 1:2],
                        op0=OP.mult, op1=OP.add,
                    )
                    nc.scalar.activation(
                        obig[:, 1, p1:hi], ybig[:, 1, p1:hi], AF.Identity,
                        bias=shf[:, 1:2], scale=scl[:, 1:2],
                    )
                    if first:
                        # split the first group's store so the DMA pipeline
                        # starts as soon as half the group is normalized
                        mid = (lo + hi) // 2
                        nc.sync.dma_start(
                            out_d[:, :, lo:mid], obig[:, :, lo:mid]
                        )
                        nc.sync.dma_start(
                            out_d[:, :, mid:hi], obig[:, :, mid:hi]
                        )
                        first = False
                    else:
                        nc.sync.dma_start(out_d[:, :, lo:hi], obig[:, :, lo:hi])

    nc.compile()
    return nc


def _get_program(repeat: int = 1, use_collective: bool = True):
    key = (repeat, use_collective)
    if key not in _PROGRAM_CACHE:
        _PROGRAM_CACHE[key] = build_program(repeat, use_collective)
    return _PROGRAM_CACHE[key]


def prepare_inputs(x_all, W, b, gamma, beta, segment_key):
    """Host-side sharding: gap insertion, transpose, per-core slicing.

    Returns (in_maps, tok_gpos) where tok_gpos[n] is the gapped position of
    token n in the concatenated per-core output space (core = pos // L).
    """
    x_all = np.ascontiguousarray(np.asarray(x_all, dtype=np.float32))
    W = np.asarray(W, dtype=np.float32)
    b = np.asarray(b, dtype=np.float32)
    gamma = np.asarray(gamma, dtype=np.float32)
    beta = np.asarray(beta, dtype=np.float32)
    seg = np.asarray(segment_key).reshape(-1)
    n = x_all.shape[0]
    assert n == N, f"kernel hardcodes N={N}, got {n}"

    # run-length segments of the sorted key
    change = np.flatnonzero(seg[1:] != seg[:-1]) + 1
    starts = np.concatenate(([0], change))
    ends = np.concatenate((change, [n]))
    nseg = len(starts)
    assert n + GAP * (nseg + 1) <= NCORES * L, "gapped sequence does not fit"

    # gapped position of each token
    tok_gpos = np.empty(n, dtype=np.int64)
    g = GAP
    for s, e in zip(starts, ends):
        tok_gpos[s:e] = g + np.arange(e - s)
        g += (e - s) + GAP

    # gapped, transposed input with halo: xg_t[:, PAD + gpos] = x_all[n]
    total = NCORES * L
    xg = np.zeros((total + 2 * PAD, D), dtype=np.float32)
    xg[PAD + tok_gpos] = x_all
    xg_t = np.ascontiguousarray(xg.T)  # [D, total + 2*PAD]

    mask = np.zeros(total, dtype=bfloat16)
    mask[tok_gpos] = 1.0

    # weights: wmat[d, k*D + o] = W[o, d, k] -> [128, 2, K*D] bf16
    wmat = W.transpose(1, 2, 0).reshape(D, K * D)
    w_in = np.ascontiguousarray(
        wmat.reshape(2, 128, K * D).transpose(1, 0, 2).astype(bfloat16)
    )

    eps_col = np.full(128, EPS, dtype=np.float32)
    zero_col = np.zeros(128, dtype=np.float32)
    bgb = np.stack(
        [b[:128], b[128:], gamma[:128], gamma[128:], beta[:128], beta[128:],
         eps_col, zero_col],
        axis=1,
    ).astype(np.float32)
    bgb = np.ascontiguousarray(bgb)

    in_maps = []
    for c in range(NCORES):
        xc = np.ascontiguousarray(
            xg_t[:, c * L : c * L + LH]
            .reshape(2, 128, LH).transpose(1, 0, 2).astype(bfloat16)
        )
        mc = np.ascontiguousarray(
            np.broadcast_to(mask[c * L : (c + 1) * L], (128, L))
        )
        in_maps.append({"x": xc, "w": w_in, "mask": mc, "bgb": bgb})
    return in_maps, tok_gpos


def assemble_output(results, tok_gpos):
    out = np.empty((N, D), dtype=np.float32)
    core = tok_gpos // L
    loc = tok_gpos % L
    for c in range(NCORES):
        sel = core == c
        # [128, 2, L] bf16 -> [256, L] fp32
        full = (
            results[c]["out"].astype(np.float32)
            .transpose(1, 0, 2).reshape(D, L)
        )
        out[sel] = full[:, loc[sel]].T
    return out


def kernel(x_all, W, b, gamma, beta, segment_key):
    nc = _get_program()
    in_maps, tok_gpos = prepare_inputs(x_all, W, b, gamma, beta, segment_key)
    res = run_bass_kernel_spmd(nc, in_maps, list(range(NCORES)))
    return assemble_output(res.results, tok_gpos)


# revision 30
# speedup vs baseline: 1.0594x; 1.0046x over previous
"""Trainium2 Bass kernel for segment-wise Conv1d + ReLU + BatchNorm1d.

Reference computation (nn_ConvSeg):
  - x_all [32768, 256] fp32, segment_key [32768] sorted ids (<= 8 segments)
  - per-segment Conv1d (kernel K=9, zero padding 4 at segment boundaries)
  - ReLU, then BatchNorm1d over all tokens (training stats, biased var)

Strategy (v5):
  - Host inserts 4 zero rows at each segment boundary -> the ragged
    per-segment conv becomes ONE dense conv over the gapped sequence.
  - The gapped sequence (8*4104 positions) is split into 8 equal chunks
    (one per NeuronCore) with a 4-position halo on each side.
  - All streamed tensors (x, w, mask, out) are bf16: rel error ~3e-3 vs
    the 2e-2 gate, matmul throughput identical to fp32r (1 cycle/row),
    and DMA bytes halve. Stats/PSUM/normalize stay fp32.
  - Data is transposed to [d, position]; conv = 18 accumulation passes of
    128x128xBS bf16 matmuls per (block, out-chunk) into PSUM. Blocks are
    7x508 + 380/104/64 columns: the shrinking tail keeps every stats
    chain except the tiny last block's hidden under remaining PE work.
  - ScalarE fuses bias + ReLU from PSUM; DVE does masked sum; ScalarE
    Square+accum gives sum-of-squares. Per-half st4 reductions are hoisted
    so only the tiny last block's stats sit on the critical path.
  - Cross-core BN stats use one AllGather (cheaper than AllReduce) of
    [128,4] sums. Both collective-adjacent DMAs are issued from the Pool
    engine so they sit in FIFO order around the collective (no cross-
    engine completion hops); Pool also tree-adds the 8 gathered copies
    and computes mean/var before a single ACT Sqrt + DVE reciprocal.
  - Normalize is split across DVE/Pool/ACT in proportion to their
    throughput, writes bf16, and DMAs out in column groups (first group
    split for an earlier store start); host upcasts, drops gap columns,
    reassembles [32768, 256] fp32.
"""

import numpy as np
from ml_dtypes import bfloat16

import concourse.bacc as bacc
import concourse.mybir as mybir
from concourse import tile
from concourse.bass_utils import run_bass_kernel_spmd

F32 = mybir.dt.float32
BF16 = mybir.dt.bfloat16
AF = mybir.ActivationFunctionType
OP = mybir.AluOpType
AX = mybir.AxisListType

N = 32768
D = 256  # d_in == d_out == 256
K = 9
PAD = K // 2
EPS = 1e-5

NCORES = 8
BSZ = [508] * 7 + [380, 104, 64]  # block widths (PSUM bank limit 512 fp32);
NB = len(BSZ)                     # shrinking tail so the last stats chain
OFF = [sum(BSZ[:b]) for b in range(NB)]  # (which gates the collective) is short
L = sum(BSZ)  # 4104 gapped positions per core
LH = L + 2 * PAD  # input columns incl. halo
GAP = 4  # zero rows inserted at each segment boundary (>= PAD)

# normalize / output column groups
_GB = [range(0, 3), range(3, 6), range(6, NB)]
GRP = [(OFF[bl[0]], OFF[bl[-1]] + BSZ[bl[-1]], bl) for bl in _GB]

_PROGRAM_CACHE: dict = {}


def build_program(repeat: int = 1, use_collective: bool = True,
                  conv_only: bool = False, collective_kind: str = "AllGather"):
    """Build + compile the SPMD Bass program (identical on all 8 cores)."""
    nc = bacc.Bacc(
        "TRN2", target_bir_lowering=False, debug=False, num_devices=NCORES
    )

    x_d = nc.declare_dram_parameter("x", [128, 2, LH], BF16, isOutput=False)
    w_d = nc.declare_dram_parameter("w", [128, 2, K * D], BF16, isOutput=False)
    m_d = nc.declare_dram_parameter("mask", [128, L], BF16, isOutput=False)
    bgb_d = nc.declare_dram_parameter("bgb", [128, 8], F32, isOutput=False)
    out_d = nc.declare_dram_parameter("out", [128, 2, L], BF16, isOutput=True)

    # x DMA chunks: a small first chunk so the PE can start early, then
    # three big ones. Block b only depends on chunk XBLK[b].
    XCH = [(0, OFF[0] + BSZ[0] + 2 * PAD),
           (OFF[0] + BSZ[0] + 2 * PAD, OFF[3] + BSZ[3] + 2 * PAD),
           (OFF[3] + BSZ[3] + 2 * PAD, OFF[6] + BSZ[6] + 2 * PAD),
           (OFF[6] + BSZ[6] + 2 * PAD, LH)]
    assert XCH[-1][0] < LH

    def stats_ops(b, oc, ysl, mt, scol, qcol, work, nc):  # noqa: C901
        j = oc * NB + b
        bs = BSZ[b]
        ym = work.tile([128, 508], F32, tag="ym")
        if b == NB - 1 and oc == 1:
            # keep ACT (slow fixed-cost Square) off the critical tail:
            # mask-mult on Pool, sum + square-sum on DVE
            nc.gpsimd.tensor_tensor(
                ym[:, :bs], ysl, mt[:, OFF[b] : OFF[b] + bs], OP.mult
            )
            nc.vector.tensor_reduce(
                scol[:, j : j + 1], ym[:, :bs], AX.X, OP.add
            )
            sq = work.tile([128, 508], F32, tag="sq")
            nc.vector.tensor_tensor(sq[:, :bs], ym[:, :bs], ym[:, :bs],
                                    OP.mult)
            nc.vector.tensor_reduce(
                qcol[:, j : j + 1], sq[:, :bs], AX.X, OP.add
            )
            return
        if b >= NB - 3:
            # trailing blocks: mask-mult on the idle Pool engine so DVE's
            # tail queue holds only the reductions
            nc.gpsimd.tensor_tensor(
                ym[:, :bs], ysl, mt[:, OFF[b] : OFF[b] + bs], OP.mult
            )
        else:
            nc.vector.tensor_tensor(
                ym[:, :bs], ysl, mt[:, OFF[b] : OFF[b] + bs], OP.mult
            )
        nc.vector.tensor_reduce(scol[:, j : j + 1], ym[:, :bs], AX.X, OP.add)
        sq = work.tile([128, 508], F32, tag="sq")
        nc.scalar.activation(
            sq[:, :bs], ym[:, :bs], AF.Square, bias=0.0, scale=1.0,
            accum_out=qcol[:, j : j + 1],
        )

    with tile.TileContext(nc) as tc:
        with (
            tc.tile_pool(name="const", bufs=1) as const,
            tc.tile_pool(name="ypool", bufs=1) as ypool,
            tc.tile_pool(name="psum", bufs=4, space="PSUM") as psum,
            tc.tile_pool(name="work", bufs=4) as work,
            tc.tile_pool(name="stats", bufs=1) as stats,
            tc.tile_pool(name="dram", bufs=2, space="DRAM") as dram,
        ):
            xt = const.tile([128, 2, LH], BF16)
            wt = const.tile([128, 2, K * D], BF16)
            mt = const.tile([128, L], BF16)
            bgbt = const.tile([128, 8], F32)
            ybig = ypool.tile([128, 2, L], F32)
            obig = ypool.tile([128, 2, L], BF16)
            scol = stats.tile([128, 2 * NB], F32)
            qcol = stats.tile([128, 2 * NB], F32)
            warm = stats.tile([128, 1], F32)

            for _ in range(repeat):
                # --- input DMAs, ordered so compute can start early ---
                lo, hi = XCH[0]
                nc.sync.dma_start(xt[:, :, lo:hi], x_d[:, :, lo:hi])
                nc.sync.dma_start(wt[:, :, 0:D], w_d[:, :, 0:D])
                nc.sync.dma_start(bgbt[:], bgb_d[:])
                # preload the single ACT table (relu/square/identity/sqrt)
                nc.scalar.activation(warm[:], bgbt[:, 6:7], AF.Sqrt)
                for k in range(1, K):  # remaining weights, one tap per DMA
                    nc.sync.dma_start(
                        wt[:, :, k * D : (k + 1) * D],
                        w_d[:, :, k * D : (k + 1) * D],
                    )
                for lo, hi in XCH[1:]:  # remaining x chunks
                    nc.sync.dma_start(xt[:, :, lo:hi], x_d[:, :, lo:hi])
                # mask, only needed by trailing stats ops
                nc.sync.dma_start(mt[:], m_d[:])

                # --- conv + relu + local BN stats ---
                st4 = stats.tile([128, 4], F32, tag="st4")
                for b in range(NB):
                    bs = BSZ[b]
                    for oc in range(2):
                        ps = psum.tile([128, 508], F32, tag="ps")
                        for k in range(K):
                            for dc in range(2):
                                nc.tensor.matmul(
                                    ps[:, :bs],
                                    wt[:, dc,
                                       k * D + oc * 128 : k * D + oc * 128 + 128],
                                    xt[:, dc, OFF[b] + k : OFF[b] + k + bs],
                                    start=(k == 0 and dc == 0),
                                    stop=(k == K - 1 and dc == 1),
                                )
                        ysl = ybig[:, oc, OFF[b] : OFF[b] + bs]
                        # y = relu(conv + bias)
                        if not conv_only and b == NB - 1 and oc == 1:
                            # last block: relu on DVE straight from PSUM, so
                            # the tail chain has no ACT hop
                            nc.vector.tensor_scalar(
                                out=ysl, in0=ps[:, :bs],
                                scalar1=bgbt[:, oc : oc + 1], scalar2=0.0,
                                op0=OP.add, op1=OP.max,
                            )
                        else:
                            nc.scalar.activation(
                                ysl, ps[:, :bs], AF.Relu,
                                bias=bgbt[:, oc : oc + 1], scale=1.0,
                            )
                        if not conv_only:
                            stats_ops(b, oc, ysl, mt, scol, qcol, work, nc)
                        # st4 halves as soon as their last block is done, so
                        # only the tiny last block's chain gates the collective
                        if not conv_only and b == NB - 1:
                            nc.vector.tensor_reduce(
                                st4[:, oc : oc + 1],
                                scol[:, oc * NB : oc * NB + NB], AX.X, OP.add,
                            )
                            nc.vector.tensor_reduce(
                                st4[:, 2 + oc : 3 + oc],
                                qcol[:, oc * NB : oc * NB + NB], AX.X, OP.add,
                            )

                if conv_only:
                    for lo, hi, _ in GRP:
                        nc.sync.dma_start(
                            out_d[:, :, lo:hi], ybig[:, :, lo:hi]
                        )
                    continue

                # --- global BN stats (AllGather + local reduce) ---
                gst = stats.tile([128, 4], F32, tag="gst")
                if use_collective and collective_kind == "AllGather":
                    cc_in = dram.tile([128, 4], F32, tag="cc_in")
                    cc_out = dram.tile([8, 128, 4], F32, tag="cc_out")
                    # Pool-issued: the collective is the next Pool op, so no
                    # cross-engine completion hop
                    nc.gpsimd.dma_start(cc_in[:], st4[:])
                    nc.gpsimd.collective_compute(
                        "AllGather",
                        OP.bypass,
                        replica_groups=[list(range(NCORES))],
                        ins=[cc_in.opt()],
                        outs=[cc_out.opt()],
                    )
                    # land gathered stats transposed: [128, 8 cores, 4 stats]
                    # issued from Pool: FIFO after the collective, no sem hop
                    g8 = stats.tile([128, 8, 4], F32, tag="g8")
                    nc.gpsimd.dma_start(g8[:], cc_out[:].transpose([1, 0, 2]))
                    # tree-add the 8 copies on Pool itself (same engine as
                    # the DMA -> only the DMA sem, no second engine wake-up)
                    h4 = stats.tile([128, 4, 4], F32, tag="h4")
                    h2 = stats.tile([128, 2, 4], F32, tag="h2")
                    nc.gpsimd.tensor_tensor(
                        h4[:], g8[:, 0:4, :], g8[:, 4:8, :], OP.add
                    )
                    nc.gpsimd.tensor_tensor(
                        h2[:], h4[:, 0:2, :], h4[:, 2:4, :], OP.add
                    )
                    nc.gpsimd.tensor_tensor(
                        gst[:], h2[:, 0, :], h2[:, 1, :], OP.add
                    )
                elif use_collective:
                    cc_in = dram.tile([128, 4], F32, tag="cc_in")
                    cc_out = dram.tile([128, 4], F32, tag="cc_out")
                    nc.sync.dma_start(cc_in[:], st4[:])
                    nc.gpsimd.collective_compute(
                        "AllReduce",
                        OP.add,
                        replica_groups=[list(range(NCORES))],
                        ins=[cc_in.opt()],
                        outs=[cc_out.opt()],
                    )
                    nc.sync.dma_start(gst[:], cc_out[:])
                else:
                    nc.vector.tensor_scalar_mul(gst[:], st4[:], float(NCORES))

                # --- finalize: scale = gamma*rsqrt(var+eps),
                #     shift = beta - mean*scale ---
                m4 = stats.tile([128, 4], F32, tag="m4")  # [mean, E[y^2]]
                var = stats.tile([128, 2], F32, tag="var")
                std = stats.tile([128, 2], F32, tag="std")
                inv = stats.tile([128, 2], F32, tag="inv")
                scl = stats.tile([128, 2], F32, tag="scl")
                shf = stats.tile([128, 2], F32, tag="shf")
                nc.gpsimd.tensor_scalar_mul(m4[:], gst[:], 1.0 / N)
                nc.gpsimd.tensor_tensor(var[:], m4[:, 0:2], m4[:, 0:2], OP.mult)
                nc.gpsimd.tensor_tensor(var[:], m4[:, 2:4], var[:], OP.subtract)
                nc.scalar.activation(
                    std[:], var[:], AF.Sqrt, bias=bgbt[:, 6:7], scale=1.0
                )
                nc.vector.reciprocal(inv[:], std[:])
                nc.vector.tensor_tensor(scl[:], bgbt[:, 2:4], inv[:], OP.mult)
                nc.vector.tensor_tensor(shf[:], m4[:, 0:2], scl[:], OP.mult)
                nc.vector.tensor_tensor(shf[:], bgbt[:, 4:6], shf[:], OP.subtract)

                # --- normalize (fp32 -> bf16) + write out in column groups ---
                first = True
                for lo, hi, blocks in GRP:
                    # engine shares by throughput: DVE 0.6 / Pool 0.835 /
                    # ACT 1.08 ns per column
                    W = hi - lo
                    d0 = lo + (3 * W) // 4          # oc0: DVE then Pool
                    d1 = lo + (13 * W) // 100       # oc1: DVE, Pool, ACT
                    p1 = lo + (51 * W) // 100
                    nc.vector.tensor_scalar(
                        out=obig[:, 0, lo:d0], in0=ybig[:, 0, lo:d0],
                        scalar1=scl[:, 0:1], scalar2=shf[:, 0:1],
                        op0=OP.mult, op1=OP.add,
                    )
                    nc.gpsimd.tensor_scalar(
                        out=obig[:, 0, d0:hi], in0=ybig[:, 0, d0:hi],
                        scalar1=scl[:, 0:1], scalar2=shf[:, 0:1],
                        op0=OP.mult, op1=OP.add,
                    )
                    nc.vector.tensor_scalar(
                        out=obig[:, 1, lo:d1], in0=ybig[:, 1, lo:d1],
                        scalar1=scl[:, 1:2], scalar2=shf[:, 1:2],
                        op0=OP.mult, op1=OP.add,
                    )
                    nc.gpsimd.tensor_scalar(
                        out=obig[:, 1, d1:p1], in0=ybig[:, 1, d1:p1],
                        scalar1=scl[:, 1:2], scalar2=shf[:, 1:2],
                        op0=OP.mult, op1=OP.add,
                    )
                    nc.scalar.activation(
                        obig[:, 1, p1:hi], ybig[:, 1, p1:hi], AF.Identity,
                        bias=shf[:, 1:2], scale=scl[:, 1:2],
                    )
                    if first:
                        # split the first group's store so the DMA pipeline
                        # starts as soon as half the group is normalized
                        mid = (lo + hi) // 2
                        nc.sync.dma_start(
                            out_d[:, :, lo:mid], obig[:, :, lo:mid]
                        )
                        nc.sync.dma_start(
                            out_d[:, :, mid:hi], obig[:, :, mid:hi]
                        )
                        first = False
                    else:
                        nc.sync.dma_start(out_d[:, :, lo:hi], obig[:, :, lo:hi])

    nc.compile()
    return nc


def _get_program(repeat: int = 1, use_collective: bool = True):
    key = (repeat, use_collective)
    if key not in _PROGRAM_CACHE:
        _PROGRAM_CACHE[key] = build_program(repeat, use_collective)
    return _PROGRAM_CACHE[key]


def prepare_inputs(x_all, W, b, gamma, beta, segment_key):
    """Host-side sharding: gap insertion, transpose, per-core slicing.

    Returns (in_maps, tok_gpos) where tok_gpos[n] is the gapped position of
    token n in the concatenated per-core output space (core = pos // L).
    """
    x_all = np.ascontiguousarray(np.asarray(x_all, dtype=np.float32))
    W = np.asarray(W, dtype=np.float32)
    b = np.asarray(b, dtype=np.float32)
    gamma = np.asarray(gamma, dtype=np.float32)
    beta = np.asarray(beta, dtype=np.float32)
    seg = np.asarray(segment_key).reshape(-1)
    n = x_all.shape[0]
    assert n == N, f"kernel hardcodes N={N}, got {n}"

    # run-length segments of the sorted key
    change = np.flatnonzero(seg[1:] != seg[:-1]) + 1
    starts = np.concatenate(([0], change))
    ends = np.concatenate((change, [n]))
    nseg = len(starts)
    assert n + GAP * (nseg + 1) <= NCORES * L, "gapped sequence does not fit"

    # gapped position of each token
    tok_gpos = np.empty(n, dtype=np.int64)
    g = GAP
    for s, e in zip(starts, ends):
        tok_gpos[s:e] = g + np.arange(e - s)
        g += (e - s) + GAP

    # gapped, transposed input with halo: xg_t[:, PAD + gpos] = x_all[n]
    total = NCORES * L
    xg = np.zeros((total + 2 * PAD, D), dtype=np.float32)
    xg[PAD + tok_gpos] = x_all
    xg_t = np.ascontiguousarray(xg.T)  # [D, total + 2*PAD]

    mask = np.zeros(total, dtype=bfloat16)
    mask[tok_gpos] = 1.0

    # weights: wmat[d, k*D + o] = W[o, d, k] -> [128, 2, K*D] bf16
    wmat = W.transpose(1, 2, 0).reshape(D, K * D)
    w_in = np.ascontiguousarray(
        wmat.reshape(2, 128, K * D).transpose(1, 0, 2).astype(bfloat16)
    )

    eps_col = np.full(128, EPS, dtype=np.float32)
    zero_col = np.zeros(128, dtype=np.float32)
    bgb = np.stack(
        [b[:128], b[128:], gamma[:128], gamma[128:], beta[:128], beta[128:],
         eps_col, zero_col],
        axis=1,
    ).astype(np.float32)
    bgb = np.ascontiguousarray(bgb)

    in_maps = []
    for c in range(NCORES):
        xc = np.ascontiguousarray(
            xg_t[:, c * L : c * L + LH]
            .reshape(2, 128, LH).transpose(1, 0, 2).astype(bfloat16)
        )
        mc = np.ascontiguousarray(
            np.broadcast_to(mask[c * L : (c + 1) * L], (128, L))
        )
        in_maps.append({"x": xc, "w": w_in, "mask": mc, "bgb": bgb})
    return in_maps, tok_gpos


def assemble_output(results, tok_gpos):
    out = np.empty((N, D), dtype=np.float32)
    core = tok_gpos // L
    loc = tok_gpos % L
    for c in range(NCORES):
        sel = core == c
        # [128, 2, L] bf16 -> [256, L] fp32
        full = (
            results[c]["out"].astype(np.float32)
            .transpose(1, 0, 2).reshape(D, L)
        )
        out[sel] = full[:, loc[sel]].T
    return out


def kernel(x_all, W, b, gamma, beta, segment_key):
    nc = _get_program()
    in_maps, tok_gpos = prepare_inputs(x_all, W, b, gamma, beta, segment_key)
    res = run_bass_kernel_spmd(nc, in_maps, list(range(NCORES)))
    return assemble_output(res.results, tok_gpos)
